# revision 5
# baseline (speedup 1.0000x reference)
"""Trainium2 Bass kernel for the DIAGCN dialog-GCN problem.

Math: the reference network ends in a 7-wide classifier, and every
feature-space matrix commutes with the node-space operators (banded window
sums, speaker masks, per-node mean divisions).  So all weight matrices fold
right into W_cls on the host, collapsing the network to

    P   = x @ A                  (A = [1024, 42], the only large matmul)
    out = win(hr) + hg           (node-space ops on 7-wide blocks)

with hg/hr assembled from P's six 7-wide column groups via two banded
window passes (PE matmuls against a fixed 256x256 0/1 band matrix), speaker
masking and per-relation mean division.  The loss is a scalar reduction of
the returned logits, computed on host in float64.

Sharding: 8 cores, each owns 16 contiguous dialogs (4096 nodes).  The
banded intra-dialog graph never crosses core boundaries.
"""

import numpy as np

# ---- problem constants (hardcoded) ----
B, L = 128, 256            # dialogs, dialog length
IN, H, C = 1024, 256, 7
N = B * L                  # 32768 nodes
NCORES = 8
BD = B // NCORES           # dialogs per core = 16
NN = BD * L                # nodes per core = 4096
NT = NN // 128             # node tiles per core = 32
NG = NT // 4               # DMA groups of 4 tiles = 8
KC = IN // 128             # contraction chunks = 8
W42, W43 = 42, 43

# position-major ("parity") tile order: even tiles first, then odd tiles.
# position j < 16 -> tile 2j (first half of dialog j); j >= 16 -> tile 2(j-16)+1.
POS2TILE = np.array([2 * j for j in range(16)] + [2 * j + 1 for j in range(16)])

_RUNNER = None  # cached compiled kernel


def _build_nc():
    import concourse.bacc as bacc
    import concourse.mybir as mybir
    from concourse.tile import TileContext
    from contextlib import ExitStack

    dt = mybir.dt.float32

    nc = bacc.Bacc(
        "TRN2",
        target_bir_lowering=False,
        debug=False,
        num_devices=NCORES,
        enable_asserts=False,
    )

    # xg[g, p, tt, c, n] = x[node 512g + 128tt + n, feature 128c + p]
    xg_d = nc.dram_tensor("xg", [NG, 128, 4, KC, 128], dt, kind="ExternalInput")
    a_d = nc.dram_tensor("afold", [128, KC, W42], dt, kind="ExternalInput")
    bq_d = nc.dram_tensor("bq", [128, 4 * 128], dt, kind="ExternalInput")
    sc_d = nc.dram_tensor("sc", [128, 2 * NT], dt, kind="ExternalInput")
    cst_d = nc.dram_tensor("cst", [1, 256], dt, kind="ExternalInput")
    out_d = nc.dram_tensor("out", [128, NT, C], dt, kind="ExternalOutput")

    with TileContext(nc) as tc, ExitStack() as ctx:
        cpool = ctx.enter_context(tc.tile_pool(name="const", bufs=1))
        xpool = ctx.enter_context(tc.tile_pool(name="x", bufs=3))
        spool = ctx.enter_context(tc.tile_pool(name="s", bufs=1))
        pmm = ctx.enter_context(tc.tile_pool(name="pmm", bufs=4, space="PSUM"))
        pwin = ctx.enter_context(tc.tile_pool(name="pwin", bufs=4, space="PSUM"))

        a_sb = cpool.tile([128, KC, W42], dt)
        nc.sync.dma_start(out=a_sb, in_=a_d[:, :, :])
        bq_sb = cpool.tile([128, 4 * 128], dt)
        nc.sync.dma_start(out=bq_sb, in_=bq_d[:, :])
        sc_sb = cpool.tile([128, 2 * NT], dt)
        nc.sync.dma_start(out=sc_sb, in_=sc_d[:, :])
        cst_sb = cpool.tile([1, 256], dt)
        nc.sync.dma_start(out=cst_sb, in_=cst_d[:, :])

        tile2pos = np.argsort(POS2TILE)

        # ---- stage A: P = x @ A (+ bias row via K=1 matmul) ----
        P_all = spool.tile([128, NT, W42], dt)
        for g in range(NG):
            xt = xpool.tile([128, 4, KC, 128], dt)
            nc.sync.dma_start(out=xt, in_=xg_d[g])
            for tt in range(4):
                t = 4 * g + tt
                j = int(tile2pos[t])
                ps = pmm.tile([128, W42], dt, tag="mm")
                nc.tensor.matmul(
                    ps,
                    lhsT=cst_sb[0:1, 0:128],
                    rhs=cst_sb[0:1, 128 : 128 + W42],
                    start=True,
                    stop=False,
                )
                for c in range(KC):
                    nc.tensor.matmul(
                        ps,
                        lhsT=xt[:, tt, c, :],
                        rhs=a_sb[:, c, :],
                        start=False,
                        stop=(c == KC - 1),
                    )
                nc.scalar.copy(out=P_all[:, j, :], in_=ps)

        # ---- stage B: build window-input block [g0 r0 | s*g0 s*r0 s*g1 s*r1 | s] ----
        winin = spool.tile([128, NT, W43], dt)
        nc.vector.tensor_copy(out=winin[:, :, 0:14], in_=P_all[:, :, 14:28])
        for j in range(NT):
            nc.vector.tensor_scalar_mul(
                out=winin[:, j, 14:42],
                in0=P_all[:, j, 14:42],
                scalar1=sc_sb[:, j : j + 1],
            )
        nc.vector.tensor_copy(out=winin[:, :, 42], in_=sc_sb[:, 0:NT])

        # ---- stage C: window pass 1 (banded matmuls) ----
        Wl = spool.tile([128, NT, W43], dt)
        for h in range(2):
            ev = winin[:, 8 * h : 8 * h + 8, :]
            od = winin[:, 16 + 8 * h : 24 + 8 * h, :]
            pse = pwin.tile([128, 8 * W43], dt, tag="win")
            nc.tensor.matmul(pse, lhsT=bq_sb[:, 0:128], rhs=ev, start=True, stop=False)
            nc.tensor.matmul(pse, lhsT=bq_sb[:, 256:384], rhs=od, start=False, stop=True)
            nc.scalar.copy(out=Wl[:, 8 * h : 8 * h + 8, :], in_=pse)
            pso = pwin.tile([128, 8 * W43], dt, tag="win")
            nc.tensor.matmul(pso, lhsT=bq_sb[:, 128:256], rhs=ev, start=True, stop=False)
            nc.tensor.matmul(pso, lhsT=bq_sb[:, 384:512], rhs=od, start=False, stop=True)
            nc.scalar.copy(out=Wl[:, 16 + 8 * h : 24 + 8 * h, :], in_=pso)

        # ---- stage D: masks, counts, means, assemble hg/hr ----
        M = spool.tile([128, NT, 29], dt)
        for j in range(NT):
            nc.vector.tensor_scalar_mul(
                out=M[:, j, :], in0=Wl[:, j, 14:43], scalar1=sc_sb[:, j : j + 1]
            )
        U = spool.tile([128, NT, 14], dt)
        nc.vector.tensor_sub(out=U, in0=Wl[:, :, 0:14], in1=M[:, :, 0:14])
        cn0 = spool.tile([128, NT], dt)
        nc.vector.tensor_sub(out=cn0, in0=sc_sb[:, NT : 2 * NT], in1=M[:, :, 28])
        cn1 = spool.tile([128, NT], dt)
        nc.vector.tensor_scalar_max(out=cn0, in0=cn0, scalar1=1.0)
        nc.vector.tensor_scalar_max(out=cn1, in0=M[:, :, 28], scalar1=1.0)
        d0 = spool.tile([128, NT], dt)
        d1 = spool.tile([128, NT], dt)
        nc.vector.reciprocal(out=d0, in_=cn0)
        nc.vector.reciprocal(out=d1, in_=cn1)
        T1 = spool.tile([128, NT, 14], dt)
        T2 = spool.tile([128, NT, 14], dt)
        for j in range(NT):
            nc.vector.tensor_scalar_mul(
                out=T1[:, j, :], in0=U[:, j, :], scalar1=d0[:, j : j + 1]
            )
            nc.vector.tensor_scalar_mul(
                out=T2[:, j, :], in0=M[:, j, 14:28], scalar1=d1[:, j : j + 1]
            )
        tmp = spool.tile([128, NT, 14], dt)
        nc.vector.tensor_add(out=tmp, in0=T1, in1=T2)
        HG = spool.tile([128, NT, C], dt)
        HR = spool.tile([128, NT, C], dt)
        nc.vector.tensor_add(out=HG, in0=P_all[:, :, 0:7], in1=tmp[:, :, 0:7])
        nc.vector.tensor_add(out=HR, in0=P_all[:, :, 7:14], in1=tmp[:, :, 7:14])

        # ---- stage E: window pass 2 + final add ----
        out_sb = spool.tile([128, NT, C], dt)
        for h in range(2):
            ev = HR[:, 8 * h : 8 * h + 8, :]
            od = HR[:, 16 + 8 * h : 24 + 8 * h, :]
            pse = pwin.tile([128, 8 * C], dt, tag="win")
            nc.tensor.matmul(pse, lhsT=bq_sb[:, 0:128], rhs=ev, start=True, stop=False)
            nc.tensor.matmul(pse, lhsT=bq_sb[:, 256:384], rhs=od, start=False, stop=True)
            nc.vector.tensor_add(
                out=out_sb[:, 8 * h : 8 * h + 8, :],
                in0=HG[:, 8 * h : 8 * h + 8, :],
                in1=pse,
            )
            pso = pwin.tile([128, 8 * C], dt, tag="win")
            nc.tensor.matmul(pso, lhsT=bq_sb[:, 128:256], rhs=ev, start=True, stop=False)
            nc.tensor.matmul(pso, lhsT=bq_sb[:, 384:512], rhs=od, start=False, stop=True)
            nc.vector.tensor_add(
                out=out_sb[:, 16 + 8 * h : 24 + 8 * h, :],
                in0=HG[:, 16 + 8 * h : 24 + 8 * h, :],
                in1=pso,
            )
        nc.sync.dma_start(out=out_d[:, :, :], in_=out_sb)

    nc.compile()
    return nc


def _make_runner():
    """Compile once; return run(in_maps) -> list per-core output dicts,
    plus hooks for timing."""
    import jax
    import numpy as _np
    from jax.sharding import Mesh, PartitionSpec
    from jax.experimental.shard_map import shard_map
    from concourse import bass2jax
    import concourse.mybir as mybir

    nc = _build_nc()
    bass2jax.install_neuronx_cc_hook()

    partition_name = nc.partition_id_tensor.name if nc.partition_id_tensor else None
    in_names, out_names, out_avals, zero_outs = [], [], [], []
    for alloc in nc.m.functions[0].allocations:
        if not isinstance(alloc, mybir.MemoryLocationSet):
            continue
        name = alloc.memorylocations[0].name
        if alloc.kind == "ExternalInput":
            if name != partition_name:
                in_names.append(name)
        elif alloc.kind == "ExternalOutput":
            shape = tuple(alloc.tensor_shape)
            dtype = mybir.dt.np(alloc.dtype)
            out_names.append(name)
            out_avals.append(jax.core.ShapedArray(shape, dtype))
            zero_outs.append(_np.zeros(shape, dtype))
    n_params = len(in_names)
    n_outs = len(out_avals)
    all_in_names = list(in_names) + list(out_names)
    if partition_name is not None:
        all_in_names.append(partition_name)

    def _body(*args):
        operands = list(args)
        if partition_name is not None:
            operands.append(bass2jax.partition_id_tensor())
        outs = bass2jax._bass_exec_p.bind(
            *operands,
            out_avals=tuple(out_avals),
            in_names=tuple(all_in_names),
            out_names=tuple(out_names),
            lowering_input_output_aliases=(),
            sim_require_finite=True,
            sim_require_nnan=True,
            nc=nc,
        )
        return tuple(outs)

    devices = jax.devices()[:NCORES]
    mesh = Mesh(_np.asarray(devices), ("core",))
    in_specs = (PartitionSpec("core"),) * (n_params + n_outs)
    out_specs = (PartitionSpec("core"),) * n_outs
    donate = tuple(range(n_params, n_params + n_outs))
    sharded = jax.jit(
        shard_map(_body, mesh=mesh, in_specs=in_specs, out_specs=out_specs,
                  check_rep=False),
        donate_argnums=donate,
        keep_unused=True,
    )



    def run(in_maps):
        concat_in = [
            _np.concatenate([_np.asarray(in_maps[c][name]) for c in range(NCORES)],
                            axis=0)
            for name in in_names
        ]
        concat_zeros = [
            _np.zeros((NCORES * z.shape[0], *z.shape[1:]), z.dtype) for z in zero_outs
        ]
        out_arrs = sharded(*concat_in, *concat_zeros)
        return [
            {
                name: _np.asarray(out_arrs[i]).reshape(NCORES, *out_avals[i].shape)[c]
                for i, name in enumerate(out_names)
            }
            for c in range(NCORES)
        ]

    def time_iters(in_maps, iters=32):
        """Pipeline `iters` executions back-to-back, return avg seconds/iter."""
        import time as _time

        concat_in = [
            _np.concatenate([_np.asarray(in_maps[c][name]) for c in range(NCORES)],
                            axis=0)
            for name in in_names
        ]
        dev_in = [jax.device_put(a) for a in concat_in]
        zsets = [
            [
                jax.device_put(
                    _np.zeros((NCORES * z.shape[0], *z.shape[1:]), z.dtype)
                )
                for z in zero_outs
            ]
            for _ in range(iters)
        ]
        # warmup
        r = sharded(*dev_in, *[jax.device_put(
            _np.zeros((NCORES * z.shape[0], *z.shape[1:]), z.dtype)) for z in zero_outs])
        jax.block_until_ready(r)
        t0 = _time.perf_counter()
        last = None
        for k in range(iters):
            last = sharded(*dev_in, *zsets[k])
        jax.block_until_ready(last)
        t1 = _time.perf_counter()
        return (t1 - t0) / iters

    return nc, run, time_iters


def _get_runner():
    global _RUNNER
    if _RUNNER is None:
        _RUNNER = _make_runner()
    return _RUNNER


def _prep_inputs(inputs):
    """Host-side: fold weights, shard + lay out per-core device inputs."""
    x = np.asarray(inputs["input"], dtype=np.float32)
    spk = np.asarray(inputs["speakers"]).astype(np.float32)

    f64 = lambda k: np.asarray(inputs[k], dtype=np.float64)
    W_rgcn, W_root, b_rgcn = f64("W_rgcn"), f64("W_root"), f64("b_rgcn")
    W_rel, b_rel, W_grt = f64("W_rel"), f64("b_rel"), f64("W_grt")
    W_skip, b_skip = f64("W_skip"), f64("b_skip")
    W_cls, b_cls = f64("W_cls"), f64("b_cls")

    Wg7 = W_grt @ W_cls
    Wr7 = W_rel @ W_cls
    A = np.concatenate(
        [
            W_root @ Wg7 + W_skip @ W_cls,  # q1
            W_root @ Wr7,                   # q2
            W_rgcn[0] @ Wg7,                # g0
            W_rgcn[0] @ Wr7,                # r0
            W_rgcn[1] @ Wg7,                # g1
            W_rgcn[1] @ Wr7,                # r1
        ],
        axis=1,
    ).astype(np.float32)
    cg = (b_rgcn @ Wg7 + (b_rel + b_skip) @ W_cls + b_cls).astype(np.float32)
    cr = (b_rgcn @ Wr7).astype(np.float32)

    a_host = np.ascontiguousarray(A.reshape(KC, 128, W42).transpose(1, 0, 2))

    # band matrix blocks as lhsT [k, m]
    kk = np.arange(L)
    band = (np.abs(kk[:, None] - kk[None, :]) <= 4).astype(np.float32)
    bq = np.concatenate(
        [band[:128, :128], band[:128, 128:], band[128:, :128], band[128:, 128:]],
        axis=1,
    )
    bq = np.ascontiguousarray(bq)

    cst = np.zeros((1, 256), np.float32)
    cst[0, :128] = 1.0
    cst[0, 128:135] = cg
    cst[0, 135:142] = cr

    pos = np.arange(L)
    deg_pat = (np.minimum(pos, 4) + np.minimum(L - 1 - pos, 4) + 1).astype(np.float32)

    in_maps = []
    for cidx in range(NCORES):
        base = cidx * NN
        xc = x[base : base + NN]
        xg = np.ascontiguousarray(
            xc.reshape(NG, 4, 128, KC, 128).transpose(0, 4, 1, 3, 2)
        )
        spc = spk[base : base + NN].reshape(NT, 128).T          # [128, NT] tile-major
        degc = np.tile(deg_pat, BD).reshape(NT, 128).T           # [128, NT]
        sc = np.empty((128, 2 * NT), np.float32)
        sc[:, :NT] = spc[:, POS2TILE]                            # position-major
        sc[:, NT:] = degc[:, POS2TILE]
        in_maps.append(
            {"xg": xg, "afold": a_host, "bq": bq, "sc": sc, "cst": cst}
        )
    return in_maps


def _assemble(results):
    out = np.empty((N, C), np.float32)
    for cidx in range(NCORES):
        arr = results[cidx]["out"]                # [128, NT, 7] position-major
        tiles = np.empty((NT, 128, C), np.float32)
        tiles[POS2TILE] = arr.transpose(1, 0, 2)  # undo parity ordering
        out[cidx * NN : (cidx + 1) * NN] = tiles.reshape(NN, C)
    return out


def _loss(out, labels):
    o = out.astype(np.float64)
    m = o - o.max(axis=1, keepdims=True)
    logp = m - np.log(np.exp(m).sum(axis=1, keepdims=True))
    return np.float32(-logp[np.arange(N), np.asarray(labels)].mean())


def kernel(**inputs):
    _, run, _ = _get_runner()
    in_maps = _prep_inputs(inputs)
    results = run(in_maps)
    out = _assemble(results)
    loss = _loss(out, inputs["labels"])
    return out, loss


def kernel_timed(iters=32, **inputs):
    """Returns (out, loss, avg_seconds_per_device_iteration)."""
    _, run, time_iters = _get_runner()
    in_maps = _prep_inputs(inputs)
    results = run(in_maps)
    out = _assemble(results)
    loss = _loss(out, inputs["labels"])
    dt = time_iters(in_maps, iters=iters)
    return out, loss, dt
